# revision 3
# baseline (speedup 1.0000x reference)
"""Luong attention TRN2 Bass kernel.

Full inputs: query [8,2048,512] f32, values [8,2048,512] f32,
attention_mask [8,2048] int32 (all ones), W [512,512] f32.
Returns (context [8,2048,512] f32, attn [8,2048,2048] f32), matching the
reference `(context, attn)` tuple.

Sharding: data-parallel over batch B=8 -> one batch element per NeuronCore.

Per-core pipeline (Q=S=2048, D=E=512):
  phase A: load W, values (cast f32->f32r); PE-transpose v -> vT; stream
           query in 512-row chunks, PE-transpose -> qT, matmul tqT = W^T qT.
  phase B: per 128-query tile: scores = tqT^T @ vT (f32r matmuls, PSUM f32),
           rowmax (DVE), exp with bias=-max and accumulated row sum (ACT),
           normalize -> attn out; PE-transpose unnormalized exp tiles,
           context = exp^T^T @ v accumulated over s, scaled by 1/Z.
"""

import sys

sys.path.insert(0, "/opt/trn_rl_repo")

import numpy as np

import concourse.bass as bass
import concourse.mybir as mybir
from concourse import bacc
from concourse.bass_utils import run_bass_kernel_spmd
from concourse.masks import make_identity
from concourse.tile import TileContext

FP32 = mybir.dt.float32
FP32R = mybir.dt.float32r

B, Q, S, D, E = 8, 2048, 2048, 512, 512
QT = Q // 128  # 16 query tiles
ST = S // 128  # 16 key tiles
KC = D // 128  # 4 contraction chunks of 128
NC_CHUNK = 512  # matmul moving-dim tile


def build():
    nc = bacc.Bacc("TRN2", target_bir_lowering=False)
    q_h = nc.dram_tensor("q", [Q, D], FP32, kind="ExternalInput")
    v_h = nc.dram_tensor("v", [S, E], FP32, kind="ExternalInput")
    w_h = nc.dram_tensor("w", [D, E], FP32, kind="ExternalInput")
    attn_h = nc.dram_tensor("attn", [Q, S], FP32R, kind="ExternalOutput")
    ctx_h = nc.dram_tensor("ctx", [Q, E], FP32, kind="ExternalOutput")

    with TileContext(nc) as tc:
        with (
            tc.tile_pool(name="persist", bufs=1) as pp,
            tc.tile_pool(name="work", bufs=2) as wp,
            tc.tile_pool(name="stats", bufs=4) as sp,
            tc.tile_pool(name="ps_s", bufs=1, space="PSUM") as ps_s,
            tc.tile_pool(name="ps_t", bufs=2, space="PSUM") as ps_t,
            tc.tile_pool(name="ps_c", bufs=1, space="PSUM") as ps_c,
            tc.tile_pool(name="ps_q", bufs=1, space="PSUM") as ps_q,
        ):
            ident_f = pp.tile([128, 128], FP32, tag="ident_f")
            make_identity(nc, ident_f[:])
            ident = pp.tile([128, 128], FP32R, tag="ident")
            nc.vector.tensor_copy(ident[:], ident_f[:])

            # ---- phase A: weights + values resident ----
            w_sb = pp.tile([128, KC, E], FP32R, tag="w")  # [ki, ko, e]
            nc.gpsimd.dma_start(
                w_sb[:], w_h[:].rearrange("(ko ki) e -> ki ko e", ki=128)
            )
            v_sb = pp.tile([128, ST, E], FP32R, tag="v")  # [si, so, e]
            nc.gpsimd.dma_start(
                v_sb[:], v_h[:].rearrange("(so si) e -> si so e", si=128)
            )

            # vT[e, s]: transpose each [128 s,128 e] block of v
            vT_sb = pp.tile([128, KC, S], FP32R, tag="vT")
            for j in range(ST):
                for c in range(KC):
                    pt = ps_t.tile([128, 128], FP32R, tag="tr")
                    nc.tensor.transpose(
                        pt[:], v_sb[:, j, 128 * c : 128 * (c + 1)], ident[:]
                    )
                    nc.vector.tensor_copy(
                        vT_sb[:, c, 128 * j : 128 * (j + 1)], pt[:]
                    )

            # qT + tqT, streamed over 4 chunks of 512 query rows
            tqT_sb = pp.tile([128, KC, Q], FP32R, tag="tqT")  # [ei, m, q]
            for qc in range(4):
                q_tmp = wp.tile([128, 4, D], FP32R, tag="q_tmp")
                nc.gpsimd.dma_start(
                    q_tmp[:],
                    q_h[512 * qc : 512 * (qc + 1), :].rearrange(
                        "(ro ri) d -> ri ro d", ri=128
                    ),
                )
                qT_tmp = wp.tile([128, KC, 512], FP32R, tag="qT_tmp")
                for r in range(4):
                    for k in range(KC):
                        pt = ps_t.tile([128, 128], FP32R, tag="tr")
                        nc.tensor.transpose(
                            pt[:], q_tmp[:, r, 128 * k : 128 * (k + 1)], ident[:]
                        )
                        nc.vector.tensor_copy(
                            qT_tmp[:, k, 128 * r : 128 * (r + 1)], pt[:]
                        )
                for m in range(4):
                    ptq = ps_q.tile([128, 512], FP32, tag="tq")
                    for k in range(KC):
                        nc.tensor.matmul(
                            ptq[:],
                            w_sb[:, k, 128 * m : 128 * (m + 1)],
                            qT_tmp[:, k, :],
                            start=(k == 0),
                            stop=(k == KC - 1),
                        )
                    nc.vector.tensor_copy(
                        tqT_sb[:, m, 512 * qc : 512 * (qc + 1)], ptq[:]
                    )

            # ---- phase B: per 128-query tile ----
            for i in range(QT):
                psc = ps_s.tile([128, S], FP32, tag="scores")
                for j in range(4):
                    for k in range(KC):
                        nc.tensor.matmul(
                            psc[:, 512 * j : 512 * (j + 1)],
                            tqT_sb[:, k, 128 * i : 128 * (i + 1)],
                            vT_sb[:, k, 512 * j : 512 * (j + 1)],
                            start=(k == 0),
                            stop=(k == KC - 1),
                        )
                mx = sp.tile([128, 1], FP32, tag="mx")
                nc.vector.reduce_max(mx[:], psc[:], axis=mybir.AxisListType.X)
                nmx = sp.tile([128, 1], FP32, tag="nmx")
                nc.vector.tensor_scalar_mul(nmx[:], mx[:], -1.0)

                eattn = wp.tile([128, S], FP32R, tag="eattn")
                zsum = sp.tile([128, 1], FP32, tag="z")
                nc.scalar.activation(
                    eattn[:],
                    psc[:],
                    mybir.ActivationFunctionType.Exp,
                    bias=nmx[:],
                    accum_out=zsum[:],
                )
                rz = sp.tile([128, 1], FP32, tag="rz")
                nc.vector.reciprocal(rz[:], zsum[:])

                attn_out = wp.tile([128, S], FP32R, tag="attn_out")
                nc.vector.tensor_scalar_mul(attn_out[:], eattn[:], rz[:])
                nc.sync.dma_start(attn_h[128 * i : 128 * (i + 1), :], attn_out[:])

                attnT = wp.tile([128, ST, 128], FP32R, tag="attnT")
                for j in range(ST):
                    pt = ps_t.tile([128, 128], FP32R, tag="tr")
                    nc.tensor.transpose(
                        pt[:], eattn[:, 128 * j : 128 * (j + 1)], ident[:]
                    )
                    nc.vector.tensor_copy(attnT[:, j, :], pt[:])

                pctx = ps_c.tile([128, E], FP32, tag="ctx")
                for j in range(ST):
                    nc.tensor.matmul(
                        pctx[:],
                        attnT[:, j, :],
                        v_sb[:, j, :],
                        start=(j == 0),
                        stop=(j == ST - 1),
                    )
                ctx_sb = wp.tile([128, E], FP32, tag="ctx_sb")
                nc.vector.tensor_scalar_mul(ctx_sb[:], pctx[:], rz[:])
                nc.sync.dma_start(ctx_h[128 * i : 128 * (i + 1), :], ctx_sb[:])

    nc.finalize()
    return nc


_NC_CACHE = None


def kernel(query, values, attention_mask, W):
    global _NC_CACHE
    assert np.all(attention_mask != 0), "kernel assumes all-ones attention mask"
    if _NC_CACHE is None:
        _NC_CACHE = build()
    nc = _NC_CACHE

    query = np.ascontiguousarray(query, dtype=np.float32)
    values = np.ascontiguousarray(values, dtype=np.float32)
    W = np.ascontiguousarray(W, dtype=np.float32)

    in_maps = [
        {"q": query[b], "v": values[b], "w": W} for b in range(B)
    ]
    res = run_bass_kernel_spmd(nc, in_maps, core_ids=list(range(B)))
    context = np.stack([res.results[b]["ctx"] for b in range(B)])
    attn = np.stack([res.results[b]["attn"] for b in range(B)])
    return context, attn
